# revision 1
# baseline (speedup 1.0000x reference)
"""Gated linear attention kernel for one TRN2 chip (8 NeuronCores).

Math (see reference):
    q = elu(X Wq)+1, k = elu(X Wk)+1, v = X Wv, g = X Wg
    qk = sum_d(q*k) per head; k_sum = sum_seq(k); norm = sum_d(q*k_sum)
    z = qk*v/(norm+1e-6); z = LayerNorm(z)*gamma+beta; out = (z*silu(g)) Wo

Sharding: data-parallel over the 16384 tokens, 2048 per core; cores 0-3 own
batch 0, cores 4-7 batch 1.  The only cross-core coupling is k_sum (a [1,1024]
vector per batch) -> AllReduce within 4-core groups, overlapped with the
q-projection phase.

Key layout decisions (tuned against perfetto traces; 483us -> ~378us):
  * Everything the PE touches is bf16: X^T, the five weights, k, q, u.
    bf16 streams at the same 1 col/cycle as float32r but LDWEIGHTS is 2x
    faster and DMA/SBUF cost halves.  PSUM accumulation stays fp32.
  * X^T (4 MB), k (4 MB) and q (4 MB) are SBUF-resident for the whole
    kernel -- no DRAM spills, X is DMA'd exactly once.  Total HBM traffic
    is 22 MB/core (was 84 MB), so the PE never waits on DMA.
  * elu(x)+1 == min(exp(x),1) + relu(x) exactly (2 ACT + 1 DVE op); exp and
    relu live in the same ACT table so phase 1 has no table reloads.
  * The LayerNorm rsqrt is computed on the DVE (Newton iteration seeded by
    the exponent bit-hack) and -- because 1/sigma is a per-token scalar that
    commutes with the Wo matmul -- applied to the Wo *output* during PSUM
    evacuation.  Phase 2's scalar engine runs only Silu (one table),
    eliminating the per-tile Silu<->Sqrt table thrash (42 us).
  * u^T for the Wo matmul comes from the DMA XBAR hardware transpose
    (dma_start_transpose, 2-byte dtypes; ut[p,k,:] == u[:,128k+p] verified
    on hardware), keeping the 128 transposes/tile off the PE (~13 us).
  * Phase 2 is software-pipelined two tiles deep: tile t-2's transpose +
    Wo matmuls are enqueued between the v/g matmuls of tiles t-1/t, so
    Vector-queue scheduling jitter can never stall the PE.
  * The AllReduce chain runs under tc.high_priority() -- the Tile scheduler
    otherwise parks the ksum copy ~30us deep into phase 1b's vector queue,
    delaying the collective enough to stall the 1b->2 boundary.  f32
    collective only: a bf16 AllReduce returned wrong values on this stack.
  * Phase-2 PSUM pools are created py-first so the v/g accumulators land on
    banks that have been free since phase 1a, not on 1b's just-released pq
    banks (whose release waits on tile-15's elu chain).
  * Initial X^T/Wk loads are k-sliced and issued from two queues (Sync +
    Scalar) so tile 0 can start after ~1 MB; bulk prefetches use wide
    rearranged descriptors to save the ~0.6us/descriptor issue cost.
gamma is folded into Wo on the host; beta==0 is verified on the host (the
slower beta path is only built when beta is nonzero).
"""

import os

import numpy as np

import concourse.bass as bass
import concourse.mybir as mybir
import concourse.tile as tile
from concourse.bass_utils import run_bass_kernel_spmd

F32 = mybir.dt.float32
BF16 = mybir.dt.bfloat16
U32 = mybir.dt.uint32
AX = mybir.AxisListType
ALU = mybir.AluOpType
ACT_F = mybir.ActivationFunctionType

H = 1024
NH = 16
DK = 64
N_CORES = 8


def _split_multi_waits(nc, cap=1):
    """walrus in this image rejects instructions with more than ~2 sync waits
    (Tile attaches several to its kernel-tail drain).  Move excess waits onto
    preceding same-engine NoOps."""
    for f in nc.m.functions:
        for bb in f.blocks:
            insts = bb.instructions
            new_list = []
            changed = False
            for inst in insts:
                si = inst.sync_info
                waits = list(si.on_wait) if si else []
                if len(waits) > cap:
                    changed = True
                    for kk, w in enumerate(waits[:-cap]):
                        new_list.append(
                            mybir.InstNoOp(
                                name=f"{inst.name}-wsplit{kk}",
                                engine=inst.engine,
                                ins=[],
                                outs=[],
                                sync_info=mybir.SyncInfo(on_wait=[w], on_update=[]),
                            )
                        )
                    inst.sync_info = mybir.SyncInfo(
                        on_wait=waits[-cap:], on_update=list(si.on_update)
                    )
                new_list.append(inst)
            if changed:
                live = bb.instructions
                live.clear()
                for i in new_list:
                    bb.add_instruction(i)
    return nc


def build_gla(T=2048, groups=((0, 1, 2, 3), (4, 5, 6, 7)), n_devices=8,
              apply_beta=False, split_waits=True, use_silu=True):
    """Build the per-core SPMD program.  T = tokens per core."""
    assert T % 128 == 0
    NT = T // 128      # 128-token tiles
    KT = H // 128      # contraction slices

    nc = bass.Bass(num_devices=n_devices)
    xt_d = nc.declare_dram_parameter("xt", [H, T], BF16, isOutput=False)
    w_d = {
        n: nc.declare_dram_parameter(n, [H, H], BF16, isOutput=False)
        for n in ("wq", "wk", "wv", "wg", "wo")
    }
    beta_d = (
        nc.declare_dram_parameter("beta", [1, H], BF16, isOutput=False)
        if apply_beta
        else None
    )
    out_d = nc.declare_dram_parameter("out", [T, H], F32, isOutput=True)

    ks_in = nc.dram_tensor("ks_in", [1, H], F32)
    ks_out = nc.dram_tensor("ks_out", [1, H], F32)

    def mm(ps, lhsT, rhs, start, stop):
        nc.tensor.matmul(ps, lhsT=lhsT, rhs=rhs, start=start, stop=stop)

    with tile.TileContext(nc) as tc:
        with (
            tc.tile_pool(name="singles", bufs=1) as singles,
            tc.tile_pool(name="w", bufs=4) as wpool,
            tc.tile_pool(name="xt", bufs=1) as xtpool,
            tc.tile_pool(name="kt", bufs=1) as ktpool,
            tc.tile_pool(name="qt", bufs=1) as qtpool,
            tc.tile_pool(name="elu", bufs=2) as elupool,
            tc.tile_pool(name="prod", bufs=1) as prodpool,
            tc.tile_pool(name="small", bufs=3) as smpool,
            tc.tile_pool(name="z2", bufs=2) as zpool,
            # bufs=2 under apply_beta frees the 4 KB the beta broadcast needs
            tc.tile_pool(name="su", bufs=2 if apply_beta else 3) as supool,
            tc.tile_pool(name="ut", bufs=2) as utpool,
            tc.tile_pool(name="y", bufs=2) as ypool,
        ):
            ones_col = singles.tile([128, 1], BF16)
            nc.vector.memset(ones_col, 1.0)
            qk_all = singles.tile([128, NT, NH], F32)
            # rsqrt bit-hack constants (as APs: immediate ints on uint ops
            # are unreliable through the f32 immediate path)
            c_shift1 = singles.tile([128, 1], U32)
            nc.vector.memset(c_shift1, 1)
            c_magic = singles.tile([128, 1], U32)
            nc.vector.memset(c_magic, 0x5F3759DF)

            xt_all = xtpool.tile([128, KT, T], BF16)
            kt_all = ktpool.tile([128, NT, H], BF16)
            qt_all = qtpool.tile([128, NT, H], BF16)

            def alloc_w():
                return wpool.tile([128, KT, H], BF16, tag="w", name="wslot")

            def load_w(t, name, engine=None):
                # one wide descriptor: [H, H] viewed as [p, k-slice, cols]
                (engine or nc.sync).dma_start(
                    out=t[:, :, :],
                    in_=w_d[name][:, :].rearrange("(k p) n -> p k n", p=128))

            def elu1(dst, ps):
                # dst = elu(ps)+1 = min(exp(ps), 1) + relu(ps); ps is PSUM f32
                e = elupool.tile([128, 512], F32, tag="elue")
                r = elupool.tile([128, 512], F32, tag="elur")
                nc.scalar.activation(out=e, in_=ps, func=ACT_F.Exp)
                nc.scalar.activation(out=r, in_=ps, func=ACT_F.Relu)
                nc.vector.scalar_tensor_tensor(
                    out=dst, in0=e, scalar=1.0, in1=r,
                    op0=ALU.min, op1=ALU.add,
                )

            # stage the initial loads so tile 0's matmuls start after ~2 MB
            # instead of the full 8 MB X+Wk+Wq burst: X^T k-slices issue from
            # the Sync queue while Wk k-slices issue in parallel from the
            # (idle) scalar-engine queue; the bulk loads are single wide
            # descriptors (descriptor issue costs ~0.6us each).
            wk_t = alloc_w()
            for k in range(KT):
                ksl = slice(128 * k, 128 * (k + 1))
                nc.sync.dma_start(out=xt_all[:, k, 0:512],
                                  in_=xt_d[ksl, 0:512])
                nc.scalar.dma_start(out=wk_t[:, k, 0:512],
                                    in_=w_d["wk"][ksl, 0:512])
            for h in range(2):
                csl = slice(512 + 256 * h, 512 + 256 * (h + 1))
                nc.scalar.dma_start(
                    out=wk_t[:, :, csl],
                    in_=w_d["wk"][:, csl].rearrange("(k p) n -> p k n", p=128))
                nc.sync.dma_start(
                    out=xt_all[:, :, csl],
                    in_=xt_d[:, csl].rearrange("(k p) c -> p k c", p=128))
            for h in range(2):
                csl = slice(1024 + 512 * h, 1024 + 512 * (h + 1))
                nc.sync.dma_start(
                    out=xt_all[:, :, csl],
                    in_=xt_d[:, csl].rearrange("(k p) c -> p k c", p=128))
            wq_t = alloc_w()           # prefetched during phase 1a
            load_w(wq_t, "wq", engine=nc.scalar)

            # -------- phase 1a: k projection + k_sum (k kept in SBUF) ------
            with (
                tc.tile_pool(name="ks", bufs=1, space="PSUM") as kspool,
                tc.tile_pool(name="pk", bufs=2, space="PSUM") as pkpool,
            ):
                ks_ps = kspool.tile([1, H], F32)

                def emit_ksum(t):
                    for n in range(2):
                        nc.tensor.matmul(
                            ks_ps[:, 512 * n:512 * (n + 1)],
                            lhsT=ones_col,
                            rhs=kt_all[:, t, 512 * n:512 * (n + 1)],
                            start=(t == 0 and n == 0),
                            stop=(t == NT - 1 and n == 1),
                        )

                for t in range(NT):
                    for n in range(2):
                        pk = pkpool.tile([128, 512], F32, tag="pk")
                        nsl = slice(512 * n, 512 * (n + 1))
                        for k in range(KT):
                            lhs = xt_all[:, k, 128 * t:128 * (t + 1)]
                            mm(pk, lhs, wk_t[:, k, nsl], k == 0, k == KT - 1)
                        elu1(kt_all[:, t, nsl], pk)
                    # ksum of the previous tile: its elu chain finished while
                    # this tile's matmuls ran, so the PE never waits on DVE
                    if t > 0:
                        emit_ksum(t - 1)
                emit_ksum(NT - 1)
                # the AllReduce chain runs under high_priority: the Tile
                # scheduler otherwise parks the ks_sb copy ~30us deep into
                # phase 1b's vector work, which delays the collective enough
                # to stall the 1b->2 phase boundary by ~9us.
                with tc.high_priority():
                    ks_sb = singles.tile([1, H], F32)
                    nc.vector.tensor_copy(out=ks_sb, in_=ks_ps)
            with tc.high_priority():
                nc.sync.dma_start(out=ks_in[:, :], in_=ks_sb)
                nc.gpsimd.collective_compute(
                    "AllReduce", ALU.add,
                    replica_groups=[list(g) for g in groups],
                    ins=[ks_in[:, :]], outs=[ks_out[:, :]],
                )
                ksb_f32 = singles.tile([128, H], F32)
                nc.gpsimd.dma_start(out=ksb_f32,
                                    in_=ks_out[0:1, :].to_broadcast([128, H]))
                ksb = singles.tile([128, H], BF16)
                nc.gpsimd.tensor_copy(out=ksb, in_=ksb_f32)
            if apply_beta:
                beta_b = singles.tile([128, H], BF16)
                nc.gpsimd.dma_start(out=beta_b,
                                    in_=beta_d[0:1, :].to_broadcast([128, H]))

            # -------- phase 1b: q projection + qk (q kept in SBUF) ---------
            wv_t = alloc_w()           # prefetched for phase 2
            wg_t = alloc_w()
            load_w(wv_t, "wv")
            load_w(wg_t, "wg")
            with tc.tile_pool(name="pq", bufs=2, space="PSUM") as pqpool:
                for t in range(NT):
                    for n in range(2):
                        pq = pqpool.tile([128, 512], F32, tag="pq")
                        nsl = slice(512 * n, 512 * (n + 1))
                        for k in range(KT):
                            lhs = xt_all[:, k, 128 * t:128 * (t + 1)]
                            mm(pq, lhs, wq_t[:, k, nsl], k == 0, k == KT - 1)
                        elu1(qt_all[:, t, nsl], pq)
                    prod = prodpool.tile([128, H], BF16, tag="prod")
                    nc.vector.tensor_mul(prod, qt_all[:, t, :], kt_all[:, t, :])
                    nc.vector.reduce_sum(
                        out=qk_all[:, t, :],
                        in_=prod.rearrange("p (h d) -> p h d", d=DK),
                        axis=AX.X,
                    )

            # ---------------- phase 2: v, g, z, LN, gate, Wo ----------------
            wo_t = alloc_w()           # rotates into wk's slot (dead)
            load_w(wo_t, "wo")
            # pool creation order controls PSUM bank placement: py (first
            # needed ~25us into phase 2) takes the banks recycled from 1b's
            # pq pool, so the v/g matmuls (pa/pb) start on long-free banks
            # and don't wait for tile-15's elu chain to release pq.
            with (
                tc.tile_pool(name="py", bufs=2, space="PSUM") as pypool,
                tc.tile_pool(name="pa", bufs=3, space="PSUM") as papool,
                tc.tile_pool(name="pb", bufs=3, space="PSUM") as pbpool,
            ):
                def back_end(u, rsig, t):
                    # u^T via the DMA XBAR hardware transpose (2-byte dtypes
                    # only; verified layout ut[p,k,:] == u[:,128k+p]) -- keeps
                    # the 128 transposes/tile off the PE; 1/sigma is folded
                    # into the Wo PSUM->SBUF output move.
                    ut = utpool.tile([128, KT, 128], BF16, tag="ut")
                    nc.sync.dma_start_transpose(ut, u)
                    for n in range(2):
                        nsl = slice(512 * n, 512 * (n + 1))
                        py = pypool.tile([128, 512], F32, tag="py")
                        for k in range(KT):
                            mm(py, ut[:, k, :],
                               wo_t[:, k, nsl], k == 0, k == KT - 1)
                        y_sb = ypool.tile([128, 512], F32, tag="y")
                        if rsig is not None:
                            nc.vector.tensor_scalar(
                                out=y_sb, in0=py,
                                scalar1=rsig, scalar2=None, op0=ALU.mult,
                            )
                        else:
                            nc.vector.tensor_copy(out=y_sb, in_=py)
                        nc.sync.dma_start(
                            out=out_d[128 * t:128 * (t + 1), nsl], in_=y_sb)

                # 2-deep software pipeline: run tile t-2's transposes + Wo
                # while tiles t-1/t's DVE chains execute, so Vector-queue
                # scheduling jitter can never stall the PE.
                prevs = []
                for t in range(NT):
                    s_t = supool.tile([128, H], BF16, tag="s")
                    pvs = []
                    for n in range(2):
                        pv = papool.tile([128, 512], F32, tag="pa")
                        pg = pbpool.tile([128, 512], F32, tag="pb")
                        nsl = slice(512 * n, 512 * (n + 1))
                        for k in range(KT):
                            lhs = xt_all[:, k, 128 * t:128 * (t + 1)]
                            mm(pv, lhs, wv_t[:, k, nsl], k == 0, k == KT - 1)
                            mm(pg, lhs, wg_t[:, k, nsl], k == 0, k == KT - 1)
                        ssl = s_t[:, nsl]
                        if use_silu:
                            nc.scalar.activation(out=ssl, in_=pg, func=ACT_F.Silu)
                        else:  # CoreSim has no Silu table
                            nc.scalar.activation(out=ssl, in_=pg,
                                                 func=ACT_F.Sigmoid)
                            nc.vector.tensor_mul(ssl, ssl, pg)
                        pvs.append(pv)
                    # normalizer = per-head dot(q, k_sum)
                    nprod = prodpool.tile([128, H], BF16, tag="prod")
                    nc.vector.tensor_mul(nprod, qt_all[:, t, :], ksb)
                    norm = smpool.tile([128, NH], F32, tag="norm")
                    nc.vector.reduce_sum(
                        out=norm, in_=nprod.rearrange("p (h d) -> p h d", d=DK),
                        axis=AX.X,
                    )
                    rec = smpool.tile([128, NH], F32, tag="rec")
                    nc.vector.tensor_scalar_add(out=rec, in0=norm, scalar1=1e-6)
                    nc.vector.reciprocal(out=rec, in_=rec)
                    r = smpool.tile([128, NH], F32, tag="r")
                    nc.vector.tensor_mul(r, qk_all[:, t, :], rec)
                    # z = r (broadcast over d) * v
                    z = zpool.tile([128, H], BF16, tag="z")
                    for n in range(2):
                        rs = r[:, 8 * n:8 * (n + 1)]
                        r_b = bass.AP(tensor=rs.tensor, offset=rs.offset,
                                      ap=[list(rs.ap[0]), list(rs.ap[1]), [0, DK]])
                        nc.vector.tensor_tensor(
                            out=z[:, 512 * n:512 * (n + 1)],
                            in0=pvs[n], in1=r_b, op=ALU.mult,
                        )
                    # LayerNorm stats over the full 1024 features
                    st = smpool.tile([128, 2, nc.vector.BN_STATS_DIM], F32,
                                     tag="bnst")
                    for n in range(2):
                        nc.vector.bn_stats(out=st[:, n, :],
                                           in_=z[:, 512 * n:512 * (n + 1)])
                    mv = smpool.tile([128, nc.vector.BN_AGGR_DIM], F32, tag="mv")
                    nc.vector.bn_aggr(out=mv, in_=st)
                    # rsig = rsqrt(var + eps) on the DVE: exponent bit-hack
                    # seed + 2 Newton steps (max rel err ~5e-6).  Runs off the
                    # critical path; consumed only at Wo PSUM evacuation.
                    vq = smpool.tile([128, 1], F32, tag="vq")
                    nc.vector.tensor_scalar_add(out=vq, in0=mv[:, 1:2],
                                                scalar1=1e-5)
                    rsig = smpool.tile([128, 1], F32, tag="rsig")
                    nc.vector.tensor_scalar(
                        out=rsig.bitcast(U32), in0=vq.bitcast(U32),
                        scalar1=c_shift1[:, 0:1], scalar2=None,
                        op0=ALU.logical_shift_right,
                    )
                    nc.vector.tensor_tensor(
                        out=rsig.bitcast(U32), in0=c_magic,
                        in1=rsig.bitcast(U32), op=ALU.subtract,
                    )
                    nt1 = smpool.tile([128, 1], F32, tag="nt1")
                    for _ in range(2):
                        nc.vector.tensor_mul(nt1, rsig, rsig)
                        nc.vector.tensor_mul(nt1, nt1, vq)
                        nc.vector.tensor_scalar(
                            out=nt1, in0=nt1, scalar1=-0.5, scalar2=1.5,
                            op0=ALU.mult, op1=ALU.add,
                        )
                        nc.vector.tensor_mul(rsig, rsig, nt1)
                    # u = (z - mu) * silu(g); 1/sigma deferred past Wo
                    u = supool.tile([128, H], BF16, tag="u")
                    if apply_beta:
                        # beta breaks the deferral: apply rsig here instead
                        nc.vector.tensor_scalar(
                            out=u, in0=z, scalar1=mv[:, 0:1], scalar2=rsig,
                            op0=ALU.subtract, op1=ALU.mult,
                        )
                        nc.vector.tensor_add(out=u, in0=u, in1=beta_b)
                        nc.vector.tensor_mul(u, u, s_t)
                        rsig_eff = None
                    else:
                        nc.vector.tensor_scalar(
                            out=u, in0=z, scalar1=mv[:, 0:1], scalar2=None,
                            op0=ALU.subtract,
                        )
                        nc.vector.tensor_mul(u, u, s_t)
                        rsig_eff = rsig
                    prevs.append((u, rsig_eff, t))
                    if len(prevs) > 2:
                        back_end(*prevs.pop(0))
                for p in prevs:
                    back_end(*p)
    return _split_multi_waits(nc) if split_waits else nc


# ------------------------------------------------------------------
# host glue
# ------------------------------------------------------------------
_CACHE = {}
LAST_RESULT = None


def kernel(hidden_states, Wq, Wk, Wv, Wg, Wo, gamma, beta):
    import ml_dtypes
    bf16 = ml_dtypes.bfloat16

    hs = np.asarray(hidden_states, dtype=np.float32)
    Wq = np.asarray(Wq, dtype=np.float32)
    Wk = np.asarray(Wk, dtype=np.float32)
    Wv = np.asarray(Wv, dtype=np.float32)
    Wg = np.asarray(Wg, dtype=np.float32)
    Wo = np.asarray(Wo, dtype=np.float32)
    gamma = np.asarray(gamma, dtype=np.float32)
    beta = np.asarray(beta, dtype=np.float32)

    b, s, h = hs.shape
    tokens = hs.reshape(b * s, h)
    n_tok = b * s
    T = n_tok // N_CORES
    assert s % T == 0, "core token shards must not straddle batches"
    cores_per_batch = s // T

    groups = tuple(
        tuple(range(bi * cores_per_batch, (bi + 1) * cores_per_batch))
        for bi in range(b)
    )
    apply_beta = bool(np.any(beta))

    key = (T, groups, apply_beta)
    if key not in _CACHE:
        _CACHE[key] = build_gla(T=T, groups=groups, apply_beta=apply_beta)
    nc = _CACHE[key]

    wo_eff = (gamma[:, None] * Wo).astype(bf16)
    wq_b = Wq.astype(bf16)
    wk_b = Wk.astype(bf16)
    wv_b = Wv.astype(bf16)
    wg_b = Wg.astype(bf16)
    in_maps = []
    for i in range(N_CORES):
        m = {
            "xt": np.ascontiguousarray(tokens[i * T:(i + 1) * T].T).astype(bf16),
            "wq": wq_b, "wk": wk_b, "wv": wv_b, "wg": wg_b, "wo": wo_eff,
        }
        if apply_beta:
            m["beta"] = beta.reshape(1, h)
        in_maps.append(m)

    res = run_bass_kernel_spmd(
        nc, in_maps, core_ids=list(range(N_CORES)),
        trace=bool(os.environ.get("GLA_TRACE")),
    )
    global LAST_RESULT
    LAST_RESULT = res
    out = np.concatenate([res.results[i]["out"] for i in range(N_CORES)], axis=0)
    return out.reshape(b, s, h)



# revision 2
# speedup vs baseline: 1.0862x; 1.0862x over previous
"""Gated linear attention kernel for one TRN2 chip (8 NeuronCores).

Math (see reference):
    q = elu(X Wq)+1, k = elu(X Wk)+1, v = X Wv, g = X Wg
    qk = sum_d(q*k) per head; k_sum = sum_seq(k); norm = sum_d(q*k_sum)
    z = qk*v/(norm+1e-6); z = LayerNorm(z)*gamma+beta; out = (z*silu(g)) Wo

Sharding: data-parallel over the 16384 tokens, 2048 per core; cores 0-3 own
batch 0, cores 4-7 batch 1.  The only cross-core coupling is k_sum (a [1,1024]
vector per batch) -> AllReduce within 4-core groups, overlapped with the
q-projection phase.

v2 (378us -> target ~305us): the whole kernel is PE-streaming-bound at the
GPIO-throttled 1.95 GHz clock (13/16 clock-gate kicks in ~60us into the run;
MMs pipeline perfectly at 263ns/512cols), so the only lever is cutting PE
cycles:
  * The q/k projections run in fp8-e4m3 with perf_mode=DoubleRow: 2 fp8 MACs
    per cell per cycle, pairing adjacent 128-row k-slices via 3D APs
    [128, 2, M] / [128, 2, N] (verified on HW, max rel err 8e-4).  Host
    pre-scales Wq/Wk by 32 (std 1/32 -> 1) so e4m3 sees unit-scale data; the
    1/32 descale folds into the elu ACTs' scale input for free.  End-to-end
    rel err simulated on host: 6.5e-3 (vs 4.7e-3 all-bf16, gate 2e-2) -- the
    q-quantization error largely cancels between qk and norm = q.k_sum, and
    k_sum averages 8192 positive terms.  v/g/o CANNOT go fp8 (3.5e-2+ each).
  * Phase 1a/1b accumulate into [128,1024] 2-bank PSUM tiles so elu runs as
    one wide Exp + one wide Relu ACT (scale=1/32) + one stt -- keeps the
    scalar engine (2.3us/tile) under the PE (2.4us/tile).
  * qk = sum_d(q*k) moves from phase 1b to phase 2 (the DVE has ~1.3us/tile
    slack there; in 1b it would have been the 3.2us/tile bottleneck).
  * SBUF lifetimes: xt8/wq8/wk8 pools close after 1b; Wo loads into the freed
    bytes via the gpsimd queue (the scalar queue runs 1b's ACTs and would
    deadlock waiting for the space; gpsimd is idle after the AllReduce).
  * Output is written bf16 (halves the store traffic; +0.1% error), upcast
    to f32 on the host.
Carried over from v1: X^T/k/q SBUF-resident, elu(x)+1 == min(exp(x),1) +
relu(x), DVE rsqrt Newton with 1/sigma folded into the Wo PSUM evacuation,
DMA-XBAR transpose for u^T, 2-deep phase-2 software pipeline, high-priority
AllReduce chain, k-sliced initial loads.  gamma is folded into Wo on the
host; beta==0 is verified on the host.
"""

import os

import numpy as np

import concourse.bass as bass
import concourse.mybir as mybir
import concourse.tile as tile
from concourse.bass_utils import run_bass_kernel_spmd

F32 = mybir.dt.float32
BF16 = mybir.dt.bfloat16
FP8 = mybir.dt.float8e4
U32 = mybir.dt.uint32
AX = mybir.AxisListType
ALU = mybir.AluOpType
ACT_F = mybir.ActivationFunctionType
DR = mybir.MatmulPerfMode.DoubleRow

H = 1024
NH = 16
DK = 64
N_CORES = 8
WSCALE = 32.0          # host multiplies Wq/Wk by this before e4m3 quantization
ISCALE = 1.0 / WSCALE  # folded into the elu ACTs


def _split_multi_waits(nc, cap=1):
    """walrus in this image rejects instructions with more than ~2 sync waits
    (Tile attaches several to its kernel-tail drain).  Move excess waits onto
    preceding same-engine NoOps."""
    for f in nc.m.functions:
        for bb in f.blocks:
            insts = bb.instructions
            new_list = []
            changed = False
            for inst in insts:
                si = inst.sync_info
                waits = list(si.on_wait) if si else []
                if len(waits) > cap:
                    changed = True
                    for kk, w in enumerate(waits[:-cap]):
                        new_list.append(
                            mybir.InstNoOp(
                                name=f"{inst.name}-wsplit{kk}",
                                engine=inst.engine,
                                ins=[],
                                outs=[],
                                sync_info=mybir.SyncInfo(on_wait=[w], on_update=[]),
                            )
                        )
                    inst.sync_info = mybir.SyncInfo(
                        on_wait=waits[-cap:], on_update=list(si.on_update)
                    )
                new_list.append(inst)
            if changed:
                live = bb.instructions
                live.clear()
                for i in new_list:
                    bb.add_instruction(i)
    return nc


def build_gla(T=2048, groups=((0, 1, 2, 3), (4, 5, 6, 7)), n_devices=8,
              apply_beta=False, split_waits=True, use_silu=True):
    """Build the per-core SPMD program.  T = tokens per core."""
    assert T % 128 == 0
    NT = T // 128      # 128-token tiles
    KT = H // 128      # contraction slices
    KP = KT // 2       # DoubleRow k-pair slices

    nc = bass.Bass(num_devices=n_devices)
    xt_d = nc.declare_dram_parameter("xt", [H, T], BF16, isOutput=False)
    xt8_d = nc.declare_dram_parameter("xt8", [H, T], FP8, isOutput=False)
    wq8_d = nc.declare_dram_parameter("wq8", [H, H], FP8, isOutput=False)
    wk8_d = nc.declare_dram_parameter("wk8", [H, H], FP8, isOutput=False)
    w_d = {
        n: nc.declare_dram_parameter(n, [H, H], BF16, isOutput=False)
        for n in ("wv", "wg", "wo")
    }
    beta_d = (
        nc.declare_dram_parameter("beta", [1, H], BF16, isOutput=False)
        if apply_beta
        else None
    )
    out_d = nc.declare_dram_parameter("out", [T, H], BF16, isOutput=True)

    ks_in = nc.dram_tensor("ks_in", [1, H], F32)
    ks_out = nc.dram_tensor("ks_out", [1, H], F32)
    ksb_dram = nc.dram_tensor("ksb16", [1, H], BF16)

    def mm(ps, lhsT, rhs, start, stop):
        nc.tensor.matmul(ps, lhsT=lhsT, rhs=rhs, start=start, stop=stop)

    with tile.TileContext(nc) as tc:
        with (
            tc.tile_pool(name="singles", bufs=1) as singles,
            tc.tile_pool(name="w", bufs=2) as wpool,
            tc.tile_pool(name="xt", bufs=1) as xtpool,
            tc.tile_pool(name="kt", bufs=1) as ktpool,
            tc.tile_pool(name="qt", bufs=1) as qtpool,
            tc.tile_pool(name="elu", bufs=2) as elupool,
            tc.tile_pool(name="prod", bufs=1) as prodpool,
            tc.tile_pool(name="small", bufs=3) as smpool,
            tc.tile_pool(name="z2", bufs=2) as zpool,
            tc.tile_pool(name="s2", bufs=2) as spool,
            tc.tile_pool(name="u2", bufs=3) as upool,
            tc.tile_pool(name="ut", bufs=2) as utpool,
            tc.tile_pool(name="y", bufs=2) as ypool,
        ):
            ones_col = singles.tile([128, 1], BF16)
            nc.vector.memset(ones_col, 1.0)
            # rsqrt bit-hack constants (as APs: immediate ints on uint ops
            # are unreliable through the f32 immediate path)
            c_shift1 = singles.tile([128, 1], U32)
            nc.vector.memset(c_shift1, 1)
            c_magic = singles.tile([128, 1], U32)
            nc.vector.memset(c_magic, 0x5F3759DF)

            xt_all = xtpool.tile([128, KT, T], BF16)
            kt_all = ktpool.tile([128, NT, H], BF16)
            qt_all = qtpool.tile([128, NT, H], BF16)
            wv_t = wpool.tile([128, KT, H], BF16, tag="w", name="wv")
            wg_t = wpool.tile([128, KT, H], BF16, tag="w", name="wg")

            def load_w(t, name, engine=None):
                # one wide descriptor: [H, H] viewed as [p, k-slice, cols]
                (engine or nc.sync).dma_start(
                    out=t[:, :, :],
                    in_=w_d[name][:, :].rearrange("(k p) n -> p k n", p=128))

            def elu1(dst, ps):
                # dst = elu(ps/32)+1 = min(exp(ps/32), 1) + relu(ps)/32;
                # ps is a 2-bank [128,1024] PSUM tile, one wide ACT each
                e = elupool.tile([128, H], BF16, tag="elue")
                r = elupool.tile([128, H], BF16, tag="elur")
                nc.scalar.activation(out=e, in_=ps, func=ACT_F.Exp, scale=ISCALE)
                nc.scalar.activation(out=r, in_=ps, func=ACT_F.Relu, scale=ISCALE)
                nc.vector.scalar_tensor_tensor(
                    out=dst, in0=e, scalar=1.0, in1=r,
                    op0=ALU.min, op1=ALU.add,
                )

            with (
                tc.tile_pool(name="x8", bufs=1) as xt8pool,
                tc.tile_pool(name="w8", bufs=2) as w8pool,
            ):
                xt8_all = xt8pool.tile([128, KT, T], FP8)
                wk8_t = w8pool.tile([128, KT, H], FP8, tag="w8", name="wk8")
                wq8_t = w8pool.tile([128, KT, H], FP8, tag="w8", name="wq8")

                # stage the initial loads so tile 0's matmuls start after ~1 MB:
                # wk8 n-halves on the scalar queue, xt8 token-strips on Sync;
                # the bulk loads are single wide rearranged descriptors.
                for k in range(KT):
                    ksl = slice(128 * k, 128 * (k + 1))
                    nc.scalar.dma_start(out=wk8_t[:, k, 0:512],
                                        in_=wk8_d[ksl, 0:512])
                    nc.sync.dma_start(out=xt8_all[:, k, 0:512],
                                      in_=xt8_d[ksl, 0:512])
                for k in range(KT):
                    ksl = slice(128 * k, 128 * (k + 1))
                    nc.scalar.dma_start(out=wk8_t[:, k, 512:1024],
                                        in_=wk8_d[ksl, 512:1024])
                for h in range(3):
                    csl = slice(512 + 512 * h, 1024 + 512 * h)
                    nc.sync.dma_start(
                        out=xt8_all[:, :, csl],
                        in_=xt8_d[:, csl].rearrange("(k p) c -> p k c", p=128))
                nc.scalar.dma_start(
                    out=wq8_t[:, :, :],
                    in_=wq8_d[:, :].rearrange("(k p) n -> p k n", p=128))
                # phase-2 inputs stream during phase 1 (Sync queue, after xt8)
                for h in range(4):
                    csl = slice(512 * h, 512 * (h + 1))
                    nc.sync.dma_start(
                        out=xt_all[:, :, csl],
                        in_=xt_d[:, csl].rearrange("(k p) c -> p k c", p=128))
                load_w(wv_t, "wv")
                load_w(wg_t, "wg")

                def dr_proj(pk, w8_t, t):
                    # contraction 1024 as 4 DoubleRow pair-slices of 256
                    for n in range(2):
                        nsl = slice(512 * n, 512 * (n + 1))
                        for s in range(KP):
                            nc.tensor.matmul(
                                pk[:, nsl],
                                lhsT=xt8_all[:, 2 * s:2 * s + 2,
                                             128 * t:128 * (t + 1)],
                                rhs=w8_t[:, 2 * s:2 * s + 2, nsl],
                                start=(s == 0), stop=(s == KP - 1),
                                perf_mode=DR,
                            )

                # ---- phase 1a: k projection + k_sum (k kept in SBUF) ------
                with (
                    tc.tile_pool(name="ks", bufs=1, space="PSUM") as kspool,
                    tc.tile_pool(name="pk", bufs=2, space="PSUM") as pkpool,
                ):
                    ks_ps = kspool.tile([1, H], F32)

                    def emit_ksum(t):
                        for n in range(2):
                            nc.tensor.matmul(
                                ks_ps[:, 512 * n:512 * (n + 1)],
                                lhsT=ones_col,
                                rhs=kt_all[:, t, 512 * n:512 * (n + 1)],
                                start=(t == 0 and n == 0),
                                stop=(t == NT - 1 and n == 1),
                            )

                    for t in range(NT):
                        pk = pkpool.tile([128, H], F32, tag="pk")
                        dr_proj(pk, wk8_t, t)
                        elu1(kt_all[:, t, :], pk)
                        # ksum of the previous tile: its elu chain finished
                        # while this tile's matmuls ran -> PE never waits
                        if t > 0:
                            emit_ksum(t - 1)
                    emit_ksum(NT - 1)
                    with tc.high_priority():
                        ks_sb = singles.tile([1, H], F32)
                        nc.vector.tensor_copy(out=ks_sb, in_=ks_ps)
                # AllReduce + broadcast chain under high_priority (the Tile
                # scheduler otherwise parks it deep in 1b's queues); all on
                # the gpsimd queue, idle during 1b.  f32 collective only: a
                # bf16 AllReduce returned wrong values on this stack.
                with tc.high_priority():
                    nc.sync.dma_start(out=ks_in[:, :], in_=ks_sb)
                    nc.gpsimd.collective_compute(
                        "AllReduce", ALU.add,
                        replica_groups=[list(g) for g in groups],
                        ins=[ks_in[:, :]], outs=[ks_out[:, :]],
                    )
                    # f32 -> bf16 via a [1,H] convert + DRAM broadcast hop
                    # (keeps the [128,H] f32 staging tile out of SBUF)
                    nc.gpsimd.dma_start(out=ks_sb, in_=ks_out[:, :])
                    ks_b16 = singles.tile([1, H], BF16)
                    nc.gpsimd.tensor_copy(out=ks_b16, in_=ks_sb)
                    nc.gpsimd.dma_start(out=ksb_dram[:, :], in_=ks_b16)
                    ksb = singles.tile([128, H], BF16)
                    nc.gpsimd.dma_start(
                        out=ksb, in_=ksb_dram[0:1, :].to_broadcast([128, H]))
                if apply_beta:
                    beta_b = singles.tile([128, H], BF16)
                    nc.gpsimd.dma_start(
                        out=beta_b, in_=beta_d[0:1, :].to_broadcast([128, H]))

                # ---- phase 1b: q projection (q kept in SBUF; qk deferred) --
                with tc.tile_pool(name="pq", bufs=2, space="PSUM") as pqpool:
                    for t in range(NT):
                        pq = pqpool.tile([128, H], F32, tag="pq")
                        dr_proj(pq, wq8_t, t)
                        elu1(qt_all[:, t, :], pq)

            # ---------------- phase 2: v, g, z, LN, gate, Wo ----------------
            with tc.tile_pool(name="wo", bufs=1) as wopool:
                # Wo lands in the bytes freed by xt8/wq8/wk8; issued from the
                # gpsimd queue (empty after the AllReduce) because the space
                # only frees once 1b's last matmul retires -- a blocked load
                # on the scalar queue would stall 1b's elu ACTs behind it.
                wo_t = wopool.tile([128, KT, H], BF16)
                nc.gpsimd.dma_start(
                    out=wo_t[:, :, :],
                    in_=w_d["wo"][:, :].rearrange("(k p) n -> p k n", p=128))
                # pool creation order controls PSUM bank placement: py (first
                # needed ~25us into phase 2) takes the banks recycled from
                # 1b's pq pool, so the v/g matmuls start on long-free banks.
                with (
                    tc.tile_pool(name="py", bufs=2, space="PSUM") as pypool,
                    tc.tile_pool(name="pa", bufs=3, space="PSUM") as papool,
                    tc.tile_pool(name="pb", bufs=3, space="PSUM") as pbpool,
                ):
                    def back_end(u, rsig, t):
                        # u^T via the DMA XBAR hardware transpose (2-byte
                        # dtypes only); 1/sigma folds into the PSUM->SBUF move
                        ut = utpool.tile([128, KT, 128], BF16, tag="ut")
                        nc.sync.dma_start_transpose(ut, u)
                        for n in range(2):
                            nsl = slice(512 * n, 512 * (n + 1))
                            py = pypool.tile([128, 512], F32, tag="py")
                            for k in range(KT):
                                mm(py, ut[:, k, :],
                                   wo_t[:, k, nsl], k == 0, k == KT - 1)
                            y_sb = ypool.tile([128, 512], BF16, tag="y")
                            if rsig is not None:
                                nc.vector.tensor_scalar(
                                    out=y_sb, in0=py,
                                    scalar1=rsig, scalar2=None, op0=ALU.mult,
                                )
                            else:
                                nc.vector.tensor_copy(out=y_sb, in_=py)
                            nc.sync.dma_start(
                                out=out_d[128 * t:128 * (t + 1), nsl],
                                in_=y_sb)

                    # 2-deep software pipeline: run tile t-2's transposes + Wo
                    # while tiles t-1/t's DVE chains execute, so Vector-queue
                    # scheduling jitter can never stall the PE.
                    prevs = []
                    for t in range(NT):
                        s_t = spool.tile([128, H], BF16, tag="s")
                        pvs = []
                        for n in range(2):
                            pv = papool.tile([128, 512], F32, tag="pa")
                            pg = pbpool.tile([128, 512], F32, tag="pb")
                            nsl = slice(512 * n, 512 * (n + 1))
                            for k in range(KT):
                                lhs = xt_all[:, k, 128 * t:128 * (t + 1)]
                                mm(pv, lhs, wv_t[:, k, nsl], k == 0, k == KT - 1)
                                mm(pg, lhs, wg_t[:, k, nsl], k == 0, k == KT - 1)
                            ssl = s_t[:, nsl]
                            if use_silu:
                                nc.scalar.activation(out=ssl, in_=pg,
                                                     func=ACT_F.Silu)
                            else:  # CoreSim has no Silu table
                                nc.scalar.activation(out=ssl, in_=pg,
                                                     func=ACT_F.Sigmoid)
                                nc.vector.tensor_mul(ssl, ssl, pg)
                            pvs.append(pv)
                        # qk = per-head dot(q, k) -- deferred from 1b
                        prod = prodpool.tile([128, H], BF16, tag="prod")
                        nc.vector.tensor_mul(prod, qt_all[:, t, :],
                                             kt_all[:, t, :])
                        qk_t = smpool.tile([128, NH], F32, tag="qk")
                        nc.vector.reduce_sum(
                            out=qk_t,
                            in_=prod.rearrange("p (h d) -> p h d", d=DK),
                            axis=AX.X,
                        )
                        # normalizer = per-head dot(q, k_sum)
                        nprod = prodpool.tile([128, H], BF16, tag="prod")
                        nc.vector.tensor_mul(nprod, qt_all[:, t, :], ksb)
                        norm = smpool.tile([128, NH], F32, tag="norm")
                        nc.vector.reduce_sum(
                            out=norm,
                            in_=nprod.rearrange("p (h d) -> p h d", d=DK),
                            axis=AX.X,
                        )
                        rec = smpool.tile([128, NH], F32, tag="rec")
                        nc.vector.tensor_scalar_add(out=rec, in0=norm,
                                                    scalar1=1e-6)
                        nc.vector.reciprocal(out=rec, in_=rec)
                        r = smpool.tile([128, NH], F32, tag="r")
                        nc.vector.tensor_mul(r, qk_t, rec)
                        # z = r (broadcast over d) * v
                        z = zpool.tile([128, H], BF16, tag="z")
                        for n in range(2):
                            rs = r[:, 8 * n:8 * (n + 1)]
                            r_b = bass.AP(tensor=rs.tensor, offset=rs.offset,
                                          ap=[list(rs.ap[0]), list(rs.ap[1]),
                                              [0, DK]])
                            nc.vector.tensor_tensor(
                                out=z[:, 512 * n:512 * (n + 1)],
                                in0=pvs[n], in1=r_b, op=ALU.mult,
                            )
                        # LayerNorm stats over the full 1024 features
                        st = smpool.tile([128, 2, nc.vector.BN_STATS_DIM], F32,
                                         tag="bnst")
                        for n in range(2):
                            nc.vector.bn_stats(out=st[:, n, :],
                                               in_=z[:, 512 * n:512 * (n + 1)])
                        mv = smpool.tile([128, nc.vector.BN_AGGR_DIM], F32,
                                         tag="mv")
                        nc.vector.bn_aggr(out=mv, in_=st)
                        # rsig = rsqrt(var + eps) on the DVE: exponent bit-hack
                        # seed + 2 Newton steps (max rel err ~5e-6).  Off the
                        # critical path; consumed only at Wo PSUM evacuation.
                        vq = smpool.tile([128, 1], F32, tag="vq")
                        nc.vector.tensor_scalar_add(out=vq, in0=mv[:, 1:2],
                                                    scalar1=1e-5)
                        rsig = smpool.tile([128, 1], F32, tag="rsig")
                        nc.vector.tensor_scalar(
                            out=rsig.bitcast(U32), in0=vq.bitcast(U32),
                            scalar1=c_shift1[:, 0:1], scalar2=None,
                            op0=ALU.logical_shift_right,
                        )
                        nc.vector.tensor_tensor(
                            out=rsig.bitcast(U32), in0=c_magic,
                            in1=rsig.bitcast(U32), op=ALU.subtract,
                        )
                        nt1 = smpool.tile([128, 1], F32, tag="nt1")
                        for _ in range(2):
                            nc.vector.tensor_mul(nt1, rsig, rsig)
                            nc.vector.tensor_mul(nt1, nt1, vq)
                            nc.vector.tensor_scalar(
                                out=nt1, in0=nt1, scalar1=-0.5, scalar2=1.5,
                                op0=ALU.mult, op1=ALU.add,
                            )
                            nc.vector.tensor_mul(rsig, rsig, nt1)
                        # u = (z - mu) * silu(g); 1/sigma deferred past Wo
                        u = upool.tile([128, H], BF16, tag="u")
                        if apply_beta:
                            # beta breaks the deferral: apply rsig here
                            nc.vector.tensor_scalar(
                                out=u, in0=z, scalar1=mv[:, 0:1], scalar2=rsig,
                                op0=ALU.subtract, op1=ALU.mult,
                            )
                            nc.vector.tensor_add(out=u, in0=u, in1=beta_b)
                            nc.vector.tensor_mul(u, u, s_t)
                            rsig_eff = None
                        else:
                            nc.vector.tensor_scalar(
                                out=u, in0=z, scalar1=mv[:, 0:1], scalar2=None,
                                op0=ALU.subtract,
                            )
                            nc.vector.tensor_mul(u, u, s_t)
                            rsig_eff = rsig
                        prevs.append((u, rsig_eff, t))
                        if len(prevs) > 2:
                            back_end(*prevs.pop(0))
                    for p in prevs:
                        back_end(*p)
    return _split_multi_waits(nc) if split_waits else nc


# ------------------------------------------------------------------
# host glue
# ------------------------------------------------------------------
_CACHE = {}
LAST_RESULT = None


def kernel(hidden_states, Wq, Wk, Wv, Wg, Wo, gamma, beta):
    import ml_dtypes
    bf16 = ml_dtypes.bfloat16
    e4m3 = ml_dtypes.float8_e4m3

    hs = np.asarray(hidden_states, dtype=np.float32)
    Wq = np.asarray(Wq, dtype=np.float32)
    Wk = np.asarray(Wk, dtype=np.float32)
    Wv = np.asarray(Wv, dtype=np.float32)
    Wg = np.asarray(Wg, dtype=np.float32)
    Wo = np.asarray(Wo, dtype=np.float32)
    gamma = np.asarray(gamma, dtype=np.float32)
    beta = np.asarray(beta, dtype=np.float32)

    b, s, h = hs.shape
    tokens = hs.reshape(b * s, h)
    n_tok = b * s
    T = n_tok // N_CORES
    assert s % T == 0, "core token shards must not straddle batches"
    cores_per_batch = s // T

    groups = tuple(
        tuple(range(bi * cores_per_batch, (bi + 1) * cores_per_batch))
        for bi in range(b)
    )
    apply_beta = bool(np.any(beta))

    key = (T, groups, apply_beta)
    if key not in _CACHE:
        _CACHE[key] = build_gla(T=T, groups=groups, apply_beta=apply_beta)
    nc = _CACHE[key]

    wo_eff = (gamma[:, None] * Wo).astype(bf16)
    wq8 = (Wq * WSCALE).astype(e4m3)
    wk8 = (Wk * WSCALE).astype(e4m3)
    wv_b = Wv.astype(bf16)
    wg_b = Wg.astype(bf16)
    in_maps = []
    for i in range(N_CORES):
        xt_f32 = np.ascontiguousarray(tokens[i * T:(i + 1) * T].T)
        m = {
            "xt": xt_f32.astype(bf16),
            "xt8": xt_f32.astype(e4m3),
            "wq8": wq8, "wk8": wk8,
            "wv": wv_b, "wg": wg_b, "wo": wo_eff,
        }
        if apply_beta:
            m["beta"] = beta.reshape(1, h).astype(bf16)
        in_maps.append(m)

    res = run_bass_kernel_spmd(
        nc, in_maps, core_ids=list(range(N_CORES)),
        trace=bool(os.environ.get("GLA_TRACE")),
    )
    global LAST_RESULT
    LAST_RESULT = res
    out = np.concatenate(
        [res.results[i]["out"].astype(np.float32) for i in range(N_CORES)],
        axis=0)
    return out.reshape(b, s, h)


# revision 15
# speedup vs baseline: 1.1708x; 1.0779x over previous
"""Gated linear attention kernel for one TRN2 chip (8 NeuronCores).

Math (see reference):
    q = elu(X Wq)+1, k = elu(X Wk)+1, v = X Wv, g = X Wg
    qk = sum_d(q*k) per head; k_sum = sum_seq(k); norm = sum_d(q*k_sum)
    z = qk*v/(norm+1e-6); z = LayerNorm(z)*gamma+beta; out = (z*silu(g)) Wo

Sharding: data-parallel over the 16384 tokens, 2048 per core; cores 0-3 own
batch 0, cores 4-7 batch 1.  The only cross-core coupling is k_sum (a [1,1024]
vector per batch) -> AllReduce within 4-core groups.

The kernel is PE-streaming-bound at the GPIO-throttled 1.95 GHz clock (the
13/16 clock-gate engages ~60us in; MMs pipeline at 263ns/512cols), so v2/v3
cut PE cycles and then keep every other engine strictly under the PE:
  * q/k projections in fp8-e4m3 perf_mode=DoubleRow: 2 fp8 MACs/cell/cycle,
    pairing adjacent 128-row k-slices via 3D APs [128,2,M]/[128,2,N]
    (HW-verified, 8e-4).  Host pre-scales Wq/Wk by 32; the 1/32 descale folds
    into the elu ACTs' scale operand.  End-to-end rel err 6.9e-3 (gate 2e-2);
    the q-quantization error cancels between qk and norm, and k_sum averages
    8192 positive terms.  v/g/o cannot go fp8 (3.5e-2+ each, and u underflows
    e4m3 by ~2^-13).
  * elu = min(exp(x),1)+relu(x) on [128,1024] 2-bank PSUM tiles; exp + half
    the relu on ACT (1.87us/tile), other relu half + combine on DVE
    (1.87us/tile), both under the 2.1us/tile of DR matmuls.
  * qk = sum_d(q*k) deferred to phase 2 where the DVE has slack.
  * The AllReduce takes ~45us trigger-to-done on this stack (mesh latency +
    skew + a one-time ~50us replica-group barrier).  A dummy warm-up
    AllReduce issued at kernel start absorbs the barrier; the real AR chain
    (vector hp copy -> gpsimd dma -> AR -> vector hp f32->bf16 -> gpsimd
    broadcast) fires right after 1a.  Phase 2 gives it ~34us of runway: the
    software pipeline is 4 deep and v PSUM evacuates to a 6-tile rolling
    SBUF buffer via scalar Identity, so v/g matmuls never wait on the
    ksb-gated DVE chain.
  * Phase-2 scalar ops (silu + v-evac) are hard-gated on a zero-bias AP
    derived from 1b's last tile: without it the scheduler interleaves silu
    into phase 1 and thrashes the exp<->silu ACT tables (8x1.3us reloads on
    the bottleneck engine).  Identity/relu live in every table set.
  * y = py*rsig evacuates on ACT (Identity, scale=rsig); u = (z-mu)*s in one
    stt; rsqrt uses 1 Newton step (rel err 1.7e-3 on rsig, ~0.2e-3 on out).
  * SBUF lifetimes: xt8/wq8/wk8 close after 1b; Wo + the v ring live in the
    freed bytes (Wo loads via the gpsimd queue -- a blocked load on the
    scalar queue would deadlock behind the gated silu).
  * Output is stored bf16 (halves write traffic), upcast on the host.
Carried over from v1: X^T/k/q SBUF-resident, DVE rsqrt Newton with 1/sigma
folded past Wo, DMA-XBAR transpose for u^T, k-sliced initial loads, gamma
folded into Wo on the host; beta==0 verified on the host.
"""

import os
from contextlib import ExitStack

import numpy as np

import concourse.bass as bass
import concourse.mybir as mybir
import concourse.tile as tile
from concourse.bass_utils import run_bass_kernel_spmd

F32 = mybir.dt.float32
BF16 = mybir.dt.bfloat16
FP8 = mybir.dt.float8e4
U32 = mybir.dt.uint32
AX = mybir.AxisListType
ALU = mybir.AluOpType
ACT_F = mybir.ActivationFunctionType
DR = mybir.MatmulPerfMode.DoubleRow

H = 1024
NH = 16
DK = 64
N_CORES = 8
WSCALE = 32.0          # host multiplies Wq/Wk by this before e4m3 quantization
ISCALE = 1.0 / WSCALE  # folded into the elu ACTs
PIPE = 4               # phase-2 software pipeline depth (AR runway)
VBUFS = 6              # rolling v ring tiles


def _split_multi_waits(nc, cap=1):
    """walrus in this image rejects instructions with more than ~2 sync waits
    (Tile attaches several to its kernel-tail drain).  Move excess waits onto
    preceding same-engine NoOps."""
    for f in nc.m.functions:
        for bb in f.blocks:
            insts = bb.instructions
            new_list = []
            changed = False
            for inst in insts:
                si = inst.sync_info
                waits = list(si.on_wait) if si else []
                if len(waits) > cap:
                    changed = True
                    for kk, w in enumerate(waits[:-cap]):
                        new_list.append(
                            mybir.InstNoOp(
                                name=f"{inst.name}-wsplit{kk}",
                                engine=inst.engine,
                                ins=[],
                                outs=[],
                                sync_info=mybir.SyncInfo(on_wait=[w], on_update=[]),
                            )
                        )
                    inst.sync_info = mybir.SyncInfo(
                        on_wait=waits[-cap:], on_update=list(si.on_update)
                    )
                new_list.append(inst)
            if changed:
                live = bb.instructions
                live.clear()
                for i in new_list:
                    bb.add_instruction(i)
    return nc


def build_gla(T=2048, groups=((0, 1, 2, 3), (4, 5, 6, 7)), n_devices=8,
              apply_beta=False, split_waits=True, use_silu=True):
    """Build the per-core SPMD program.  T = tokens per core."""
    assert T % 128 == 0
    NT = T // 128      # 128-token tiles
    KT = H // 128      # contraction slices
    KP = KT // 2       # DoubleRow k-pair slices

    nc = bass.Bass(num_devices=n_devices)
    xt_d = nc.declare_dram_parameter("xt", [H, T], BF16, isOutput=False)
    xt8_d = nc.declare_dram_parameter("xt8", [H, T], FP8, isOutput=False)
    wq8_d = nc.declare_dram_parameter("wq8", [H, H], FP8, isOutput=False)
    wk8_d = nc.declare_dram_parameter("wk8", [H, H], FP8, isOutput=False)
    w_d = {
        n: nc.declare_dram_parameter(n, [H, H], BF16, isOutput=False)
        for n in ("wv", "wg", "wo")
    }
    beta_d = (
        nc.declare_dram_parameter("beta", [1, H], BF16, isOutput=False)
        if apply_beta
        else None
    )
    out_d = nc.declare_dram_parameter("out", [T, H], BF16, isOutput=True)

    ks_in = nc.dram_tensor("ks_in", [1, H], F32)
    ks_out = nc.dram_tensor("ks_out", [1, H], F32)
    ksb_dram = nc.dram_tensor("ksb16", [1, H], BF16)
    arw_in = nc.dram_tensor("arw_in", [1, 8], F32)
    arw_out = nc.dram_tensor("arw_out", [1, 8], F32)

    def mm(ps, lhsT, rhs, start, stop):
        nc.tensor.matmul(ps, lhsT=lhsT, rhs=rhs, start=start, stop=stop)

    with tile.TileContext(nc) as tc:
        with (
            tc.tile_pool(name="singles", bufs=1) as singles,
            tc.tile_pool(name="w", bufs=2) as wpool,
            tc.tile_pool(name="xt", bufs=1) as xtpool,
            tc.tile_pool(name="kt", bufs=1) as ktpool,
            tc.tile_pool(name="qt", bufs=1) as qtpool,
            tc.tile_pool(name="small", bufs=3) as smpool,
        ):
            ones_col = singles.tile([128, 1], BF16)
            nc.vector.memset(ones_col, 1.0)
            # rsqrt bit-hack constants (as APs: immediate ints on uint ops
            # are unreliable through the f32 immediate path)
            c_shift1 = singles.tile([128, 1], U32)
            nc.vector.memset(c_shift1, 1)
            c_magic = singles.tile([128, 1], U32)
            nc.vector.memset(c_magic, 0x5F3759DF)

            # warm-up AllReduce: establishes the replica-group barrier +
            # CC stream (~50us, one-time) while phase 1 computes, so the
            # real k_sum AR only pays ring latency.
            arw_sb = singles.tile([1, 8], F32)
            nc.gpsimd.memset(arw_sb, 0.0)
            nc.gpsimd.dma_start(out=arw_in[:, :], in_=arw_sb)
            nc.gpsimd.collective_compute(
                "AllReduce", ALU.add,
                replica_groups=[list(g) for g in groups],
                ins=[arw_in[:, :]], outs=[arw_out[:, :]],
            )

            xt_all = xtpool.tile([128, KT, T], BF16)
            kt_all = ktpool.tile([128, NT, H], BF16)
            qt_all = qtpool.tile([128, NT, H], BF16)
            wv_t = wpool.tile([128, KT, H], BF16, tag="w", name="wv")
            wg_t = wpool.tile([128, KT, H], BF16, tag="w", name="wg")

            def load_w(t, name, engine=None):
                # one wide descriptor: [H, H] viewed as [p, k-slice, cols]
                (engine or nc.sync).dma_start(
                    out=t[:, :, :],
                    in_=w_d[name][:, :].rearrange("(k p) n -> p k n", p=128))

            def elu1(dst, ps):
                # dst = elu(ps/32)+1 = min(exp(ps/32), 1) + relu(ps/32);
                # exp + low relu half on ACT, high relu half + combine on DVE
                e = elupool.tile([128, H], BF16, tag="elue")
                r = elupool.tile([128, H], BF16, tag="elur")
                nc.scalar.activation(out=e, in_=ps, func=ACT_F.Exp,
                                     scale=ISCALE)
                nc.scalar.activation(out=r[:, 0:512], in_=ps[:, 0:512],
                                     func=ACT_F.Relu, scale=ISCALE)
                nc.vector.tensor_scalar(
                    out=r[:, 512:H], in0=ps[:, 512:H],
                    scalar1=ISCALE, scalar2=0.0, op0=ALU.mult, op1=ALU.max,
                )
                nc.vector.scalar_tensor_tensor(
                    out=dst, in0=e, scalar=1.0, in1=r,
                    op0=ALU.min, op1=ALU.add,
                )

            with (
                tc.tile_pool(name="x8", bufs=1) as xt8pool,
                tc.tile_pool(name="w8", bufs=2) as w8pool,
                tc.tile_pool(name="elu", bufs=2) as elupool,
            ):
                xt8_all = xt8pool.tile([128, KT, T], FP8)
                wk8_t = w8pool.tile([128, KT, H], FP8, tag="w8", name="wk8")
                wq8_t = w8pool.tile([128, KT, H], FP8, tag="w8", name="wq8")

                # staged initial loads, wide rearranged descriptors: tile 0
                # can start after ~1 MB (xt8+wk8 column halves); phase-2
                # inputs follow on the Sync queue.
                def xt8_load(c0, c1):
                    nc.sync.dma_start(
                        out=xt8_all[:, :, c0:c1],
                        in_=xt8_d[:, c0:c1].rearrange("(k p) c -> p k c",
                                                      p=128))

                def w8_load(t, src, c0, c1):
                    nc.scalar.dma_start(
                        out=t[:, :, c0:c1],
                        in_=src[:, c0:c1].rearrange("(k p) n -> p k n",
                                                    p=128))

                CH = min(512, T)
                xt8_load(0, CH)
                w8_load(wk8_t, wk8_d, 0, 512)
                w8_load(wk8_t, wk8_d, 512, 1024)
                for h in range(1, T // CH):
                    xt8_load(CH * h, CH * (h + 1))
                w8_load(wq8_t, wq8_d, 0, 1024)
                for h in range(T // CH):
                    csl = slice(CH * h, CH * (h + 1))
                    nc.sync.dma_start(
                        out=xt_all[:, :, csl],
                        in_=xt_d[:, csl].rearrange("(k p) c -> p k c", p=128))
                load_w(wv_t, "wv")
                load_w(wg_t, "wg")

                def dr_proj(pk, w8_t, t):
                    # contraction 1024 as 4 DoubleRow pair-slices of 256
                    for n in range(2):
                        nsl = slice(512 * n, 512 * (n + 1))
                        for s in range(KP):
                            nc.tensor.matmul(
                                pk[:, nsl],
                                lhsT=xt8_all[:, 2 * s:2 * s + 2,
                                             128 * t:128 * (t + 1)],
                                rhs=w8_t[:, 2 * s:2 * s + 2, nsl],
                                start=(s == 0), stop=(s == KP - 1),
                                perf_mode=DR,
                            )

                # ---- phase 1a: k projection + k_sum (k kept in SBUF) ------
                with (
                    tc.tile_pool(name="ks", bufs=1, space="PSUM") as kspool,
                    tc.tile_pool(name="pk", bufs=3, space="PSUM") as pkpool,
                ):
                    ks_ps = kspool.tile([1, H], F32)

                    def emit_ksum(t):
                        for n in range(2):
                            nc.tensor.matmul(
                                ks_ps[:, 512 * n:512 * (n + 1)],
                                lhsT=ones_col,
                                rhs=kt_all[:, t, 512 * n:512 * (n + 1)],
                                start=(t == 0),
                                stop=(t == NT - 1),
                            )

                    for t in range(NT):
                        pk = pkpool.tile([128, H], F32, tag="pk")
                        dr_proj(pk, wk8_t, t)
                        elu1(kt_all[:, t, :], pk)
                        # ksum of the previous tile: its elu chain finished
                        # while this tile's matmuls ran -> PE never waits
                        if t > 0:
                            emit_ksum(t - 1)
                    emit_ksum(NT - 1)
                    with tc.high_priority():
                        ks_sb = singles.tile([1, H], F32)
                        nc.vector.tensor_copy(out=ks_sb, in_=ks_ps)
                # real AR chain: all DMAs/collective on the gpsimd queue
                # (the Sync queue still drains phase-2 bulk loads), converts
                # on the vector queue under high_priority.
                with tc.high_priority():
                    nc.gpsimd.dma_start(out=ks_in[:, :], in_=ks_sb)
                    nc.gpsimd.collective_compute(
                        "AllReduce", ALU.add,
                        replica_groups=[list(g) for g in groups],
                        ins=[ks_in[:, :]], outs=[ks_out[:, :]],
                    )
                    nc.gpsimd.dma_start(out=ks_sb, in_=ks_out[:, :])
                    ks_b16 = singles.tile([1, H], BF16)
                    nc.vector.tensor_copy(out=ks_b16, in_=ks_sb)
                    nc.gpsimd.dma_start(out=ksb_dram[:, :], in_=ks_b16)
                    ksb = singles.tile([128, H], BF16)
                    nc.gpsimd.dma_start(
                        out=ksb, in_=ksb_dram[0:1, :].to_broadcast([128, H]))
                if apply_beta:
                    beta_b = singles.tile([128, H], BF16)
                    nc.gpsimd.dma_start(
                        out=beta_b, in_=beta_d[0:1, :].to_broadcast([128, H]))

                # ---- phase 1b: q projection (q kept in SBUF; qk deferred) --
                with tc.tile_pool(name="pq", bufs=2, space="PSUM") as pqpool:
                    for t in range(NT):
                        pq = pqpool.tile([128, H], F32, tag="pq")
                        dr_proj(pq, wq8_t, t)
                        elu1(qt_all[:, t, :], pq)

                # zero gate derived from 1b's last tile: phase-2 scalar ops
                # take it as bias so the scheduler cannot interleave them
                # into phase 1 (exp<->silu table thrash).
                gate0 = singles.tile([128, 1], F32)
                nc.vector.tensor_scalar(
                    out=gate0, in0=qt_all[:, NT - 1, 0:1],
                    scalar1=0.0, scalar2=None, op0=ALU.mult,
                )

            # ---------------- phase 2: v, g, z, LN, gate, Wo ----------------
            with ExitStack() as es2:
                pool2 = lambda n, b, **kw: es2.enter_context(
                    tc.tile_pool(name=n, bufs=b, **kw))
                wopool = pool2("wo", 1)
                vpool = pool2("vr", VBUFS)
                prodpool = pool2("prod", 1)
                zpool = pool2("z2", 3)
                spool = pool2("s2", PIPE + 1)
                upool = pool2("u2", PIPE + 1)
                utpool = pool2("ut", 3)
                ypool = pool2("y", 2)
                # rsig is consumed by back_end PIPE tiles later
                rspool = pool2("rs", PIPE + 1)
                # Wo + the v ring land in the bytes freed by xt8/wq8/wk8;
                # issued from the gpsimd queue (idle after the AR) because
                # the space frees only when 1b's last matmul retires -- a
                # blocked load on the scalar queue would deadlock behind the
                # gated silu.
                wo_t = wopool.tile([128, KT, H], BF16)
                nc.gpsimd.dma_start(
                    out=wo_t[:, :, :],
                    in_=w_d["wo"][:, :].rearrange("(k p) n -> p k n", p=128))
                # pool creation order controls PSUM bank placement: py (first
                # needed ~35us into phase 2) takes the banks recycled from
                # 1b's pq pool, so the v/g matmuls start on long-free banks.
                if True:
                    pypool = pool2("py", 2, space="PSUM")
                    papool = pool2("pa", 3, space="PSUM")
                    pbpool = pool2("pb", 3, space="PSUM")

                    def back_end(u, rsig, t):
                        # u^T via the DMA XBAR hardware transpose (2-byte
                        # dtypes only); y = py * 1/sigma evacuates on ACT
                        # (Identity is in every table set -- no reload).
                        ut = utpool.tile([128, KT, 128], BF16, tag="ut")
                        nc.sync.dma_start_transpose(ut, u)
                        for n in range(2):
                            nsl = slice(512 * n, 512 * (n + 1))
                            py = pypool.tile([128, 512], F32, tag="py")
                            for k in range(KT):
                                mm(py, ut[:, k, :],
                                   wo_t[:, k, nsl], k == 0, k == KT - 1)
                            y_sb = ypool.tile([128, 512], BF16, tag="y")
                            if rsig is not None:
                                nc.scalar.activation(out=y_sb, in_=py,
                                                     func=ACT_F.Identity,
                                                     scale=rsig)
                            else:
                                nc.scalar.activation(out=y_sb, in_=py,
                                                     func=ACT_F.Identity)
                            nc.sync.dma_start(
                                out=out_d[128 * t:128 * (t + 1), nsl],
                                in_=y_sb)

                    # PIPE-deep software pipeline: tile t's back_end (wo
                    # matmuls) is enqueued at tile t+PIPE, giving the AR +
                    # ksb-gated DVE chain ~34us of PE runway at phase-2 start
                    prevs = []
                    for t in range(NT):
                        s_t = spool.tile([128, H], BF16, tag="s")
                        v_sb = vpool.tile([128, H], BF16, tag="v")
                        for n in range(2):
                            pv = papool.tile([128, 512], F32, tag="pa")
                            pg = pbpool.tile([128, 512], F32, tag="pb")
                            nsl = slice(512 * n, 512 * (n + 1))
                            for k in range(KT):
                                lhs = xt_all[:, k, 128 * t:128 * (t + 1)]
                                mm(pv, lhs, wv_t[:, k, nsl], k == 0, k == KT - 1)
                                mm(pg, lhs, wg_t[:, k, nsl], k == 0, k == KT - 1)
                            ssl = s_t[:, nsl]
                            if use_silu:
                                nc.scalar.activation(out=ssl, in_=pg,
                                                     func=ACT_F.Silu,
                                                     bias=gate0[:, 0:1])
                            else:  # CoreSim has no Silu table
                                nc.scalar.activation(out=ssl, in_=pg,
                                                     func=ACT_F.Sigmoid,
                                                     bias=gate0[:, 0:1])
                                nc.vector.tensor_mul(ssl, ssl, pg)
                            # v PSUM -> SBUF ring on ACT: frees pa so the
                            # v/g matmuls never wait on the ksb-gated DVE
                            nc.scalar.activation(out=v_sb[:, nsl], in_=pv,
                                                 func=ACT_F.Identity,
                                                 bias=gate0[:, 0:1])
                        # qk = per-head dot(q, k) -- deferred from 1b
                        prod = prodpool.tile([128, H], BF16, tag="prod")
                        nc.vector.tensor_mul(prod, qt_all[:, t, :],
                                             kt_all[:, t, :])
                        qk_t = smpool.tile([128, NH], F32, tag="qk")
                        nc.vector.reduce_sum(
                            out=qk_t,
                            in_=prod.rearrange("p (h d) -> p h d", d=DK),
                            axis=AX.X,
                        )
                        # normalizer = per-head dot(q, k_sum)
                        nprod = prodpool.tile([128, H], BF16, tag="prod")
                        nc.vector.tensor_mul(nprod, qt_all[:, t, :], ksb)
                        norm = smpool.tile([128, NH], F32, tag="norm")
                        nc.vector.reduce_sum(
                            out=norm,
                            in_=nprod.rearrange("p (h d) -> p h d", d=DK),
                            axis=AX.X,
                        )
                        rec = smpool.tile([128, NH], F32, tag="rec")
                        nc.vector.tensor_scalar_add(out=rec, in0=norm,
                                                    scalar1=1e-6)
                        nc.vector.reciprocal(out=rec, in_=rec)
                        r = smpool.tile([128, NH], F32, tag="r")
                        nc.vector.tensor_mul(r, qk_t, rec)
                        # z = r (broadcast over d) * v
                        z = zpool.tile([128, H], BF16, tag="z")
                        for n in range(2):
                            rs = r[:, 8 * n:8 * (n + 1)]
                            r_b = bass.AP(tensor=rs.tensor, offset=rs.offset,
                                          ap=[list(rs.ap[0]), list(rs.ap[1]),
                                              [0, DK]])
                            nc.vector.tensor_tensor(
                                out=z[:, 512 * n:512 * (n + 1)],
                                in0=v_sb[:, 512 * n:512 * (n + 1)],
                                in1=r_b, op=ALU.mult,
                            )
                        # LayerNorm stats over the full 1024 features
                        st = smpool.tile([128, 2, nc.vector.BN_STATS_DIM], F32,
                                         tag="bnst")
                        for n in range(2):
                            nc.vector.bn_stats(out=st[:, n, :],
                                               in_=z[:, 512 * n:512 * (n + 1)])
                        mv = smpool.tile([128, nc.vector.BN_AGGR_DIM], F32,
                                         tag="mv")
                        nc.vector.bn_aggr(out=mv, in_=st)
                        # rsig = rsqrt(var + eps) on the DVE: exponent
                        # bit-hack seed + 1 Newton step (rel err ~1.7e-3,
                        # ~2e-4 on the output).  Off the critical path;
                        # consumed only at Wo PSUM evacuation.
                        vq = smpool.tile([128, 1], F32, tag="vq")
                        nc.vector.tensor_scalar_add(out=vq, in0=mv[:, 1:2],
                                                    scalar1=1e-5)
                        rsig = rspool.tile([128, 1], F32, tag="rsig")
                        nc.vector.tensor_tensor(
                            out=rsig.bitcast(U32), in0=vq.bitcast(U32),
                            in1=c_shift1, op=ALU.logical_shift_right,
                        )
                        nc.vector.tensor_tensor(
                            out=rsig.bitcast(U32), in0=c_magic,
                            in1=rsig.bitcast(U32), op=ALU.subtract,
                        )
                        nt1 = smpool.tile([128, 1], F32, tag="nt1")
                        nc.vector.tensor_mul(nt1, rsig, rsig)
                        nc.vector.tensor_mul(nt1, nt1, vq)
                        nc.vector.tensor_scalar(
                            out=nt1, in0=nt1, scalar1=-0.5, scalar2=1.5,
                            op0=ALU.mult, op1=ALU.add,
                        )
                        nc.vector.tensor_mul(rsig, rsig, nt1)
                        # u = (z - mu) * silu(g) in one stt; 1/sigma deferred
                        u = upool.tile([128, H], BF16, tag="u")
                        if apply_beta:
                            # beta breaks the deferral: apply rsig here
                            nc.vector.tensor_scalar(
                                out=u, in0=z, scalar1=mv[:, 0:1], scalar2=rsig,
                                op0=ALU.subtract, op1=ALU.mult,
                            )
                            nc.vector.tensor_add(out=u, in0=u, in1=beta_b)
                            nc.vector.tensor_mul(u, u, s_t)
                            rsig_eff = None
                        else:
                            nc.vector.scalar_tensor_tensor(
                                out=u, in0=z, scalar=mv[:, 0:1], in1=s_t,
                                op0=ALU.subtract, op1=ALU.mult,
                            )
                            rsig_eff = rsig
                        prevs.append((u, rsig_eff, t))
                        if len(prevs) > PIPE:
                            back_end(*prevs.pop(0))
                    for p in prevs:
                        back_end(*p)
    return _split_multi_waits(nc) if split_waits else nc


# ------------------------------------------------------------------
# host glue
# ------------------------------------------------------------------
_CACHE = {}
LAST_RESULT = None


def kernel(hidden_states, Wq, Wk, Wv, Wg, Wo, gamma, beta):
    import ml_dtypes
    bf16 = ml_dtypes.bfloat16
    e4m3 = ml_dtypes.float8_e4m3

    hs = np.asarray(hidden_states, dtype=np.float32)
    Wq = np.asarray(Wq, dtype=np.float32)
    Wk = np.asarray(Wk, dtype=np.float32)
    Wv = np.asarray(Wv, dtype=np.float32)
    Wg = np.asarray(Wg, dtype=np.float32)
    Wo = np.asarray(Wo, dtype=np.float32)
    gamma = np.asarray(gamma, dtype=np.float32)
    beta = np.asarray(beta, dtype=np.float32)

    b, s, h = hs.shape
    tokens = hs.reshape(b * s, h)
    n_tok = b * s
    T = n_tok // N_CORES
    assert s % T == 0, "core token shards must not straddle batches"
    cores_per_batch = s // T

    groups = tuple(
        tuple(range(bi * cores_per_batch, (bi + 1) * cores_per_batch))
        for bi in range(b)
    )
    apply_beta = bool(np.any(beta))

    key = (T, groups, apply_beta)
    if key not in _CACHE:
        _CACHE[key] = build_gla(T=T, groups=groups, apply_beta=apply_beta)
    nc = _CACHE[key]

    wo_eff = (gamma[:, None] * Wo).astype(bf16)
    wq8 = (Wq * WSCALE).astype(e4m3)
    wk8 = (Wk * WSCALE).astype(e4m3)
    wv_b = Wv.astype(bf16)
    wg_b = Wg.astype(bf16)
    in_maps = []
    for i in range(N_CORES):
        xt_f32 = np.ascontiguousarray(tokens[i * T:(i + 1) * T].T)
        m = {
            "xt": xt_f32.astype(bf16),
            "xt8": xt_f32.astype(e4m3),
            "wq8": wq8, "wk8": wk8,
            "wv": wv_b, "wg": wg_b, "wo": wo_eff,
        }
        if apply_beta:
            m["beta"] = beta.reshape(1, h).astype(bf16)
        in_maps.append(m)

    res = run_bass_kernel_spmd(
        nc, in_maps, core_ids=list(range(N_CORES)),
        trace=bool(os.environ.get("GLA_TRACE")),
    )
    global LAST_RESULT
    LAST_RESULT = res
    out = np.concatenate(
        [res.results[i]["out"].astype(np.float32) for i in range(N_CORES)],
        axis=0)
    return out.reshape(b, s, h)
